# revision 25
# baseline (speedup 1.0000x reference)
"""Trainium2 Bass kernel for masked dot-product attention.

Problem: B=16, Lq=Lk=2048, d=128, fp32.
  scores = Q @ K^T / sqrt(d); mask key positions >= valid_len with -1e6;
  attn = softmax(scores, axis=-1); out = attn @ V.

Strategy
--------
Sharding (unchanged from the earlier revision): work is split over
(batch, query-quarter): 16 batches x 4 q-chunks of 512 = 64 shards, 8 per
core. A shard's device cost is proportional to ceil(valid_len/128) key
tiles, so shards are sorted by tile count and slot s of every core runs the
8 shards ranked [8s, 8s+8); the compiled program bakes per-slot key extents
E_s = max tile count in that rank band. Every core executes an identical
instruction stream (SPMD); all per-core variation lives in the data.

Device pipeline per slot (one 512-wide q-chunk, T = E_s key tiles), all
matmul operands bf16 (rel-err budget is 2e-2; measured ~4.5e-3):
  MM1:  S^T[k,q] = (K^T tile).T @ Q^T          (PE, k-tile stationary)
  exp:  E = exp(S^T / sqrt(d))                 (ACT, PSUM->SBUF bf16,
                                                2 k-tiles per instruction)
  den:  dsum_g = (E_odd * z_odd) + E_even      (DVE scalar_tensor_tensor,
                                                one op per 2-tile group)
        pd += (z_even x ones128).T @ dsum_g    (PE, one matmul per group)
  MM2:  po += V_tile.T-layout @ E              (PE accumulate over tiles)
MM1/exp run one group ahead of MM2/den (software pipeline, psum double
buffered) so PE, ACT and DVE overlap.

Masking is exact and purely data-driven (the instruction stream never
depends on a shard's actual valid_len, only on the baked extent):
  * V rows at k >= valid_len are zeroed on the host, so MM2 ignores
    invalid keys regardless of E's (garbage) values there.
  * The denominator gates E through per-tile 0/1 validity columns z_t:
    because validity is monotone (z_even >= z_odd pointwise),
    z_even*(E_even + z_odd*E_odd) == z_even*E_even + z_odd*E_odd exactly,
    so one fused DVE op + one matmul per pair computes the gated sum. The
    z columns ride in the packed input blob; fully-padded tiles get z=0.

All slot inputs (Q^T chunk | K^T | V tile-major | w-stationaries | gate
columns, all bf16) are packed host-side into ONE DRAM blob per slot,
transferred as two DMAs: the Q|K prefix first so MM1 starts before
V/w/gates arrive. Outputs (num [128,512] fp32, den [1,512] fp32) are
copied PSUM->SBUF on DVE and DMA'd out. Host computes out = (num^T /
den).T per shard. Softmax needs no max-subtraction: scores ~ N(0,1) so
exp() cannot overflow, and invalid columns are excluded exactly.
"""

import math

import numpy as np

B, L, D = 16, 2048, 128
NCORES = 8
QCHUNK = 512
NQCHUNKS = L // QCHUNK
NSLOTS = B * NQCHUNKS // NCORES  # 8
SCALE = 1.0 / math.sqrt(D)

_programs = {}

# Test hooks: _REPEAT>1 duplicates the whole slot schedule inside one NEFF
# (python-unrolled); _LOOP_N>1 wraps the schedule in a hardware For_i loop
# (small NEFF, device time scales ~linearly -> robust wall-clock deltas).
_TRACE = False
_REPEAT = 1
_LOOP_N = 1
_last_results = None


def _blob_width(T):
    ng = (T + 1) // 2
    return 256 * T + QCHUNK + 129 * ng


def _blob_offsets(T):
    """Blob column layout: Q^T chunk and K^T first (MM1 prerequisites, DMA'd
    as the first chunk so MM1 can start before V/w/gates arrive), then V
    tile-major, then den stationaries and gate columns."""
    ng = (T + 1) // 2
    q_off = 0
    k_off = QCHUNK
    v_off = QCHUNK + 128 * T
    w_off = v_off + 128 * T
    g_off = w_off + 128 * ng
    return q_off, k_off, v_off, w_off, g_off


def _build_program(extents, repeat=1, loop_n=1, staggered=False, body_mode="full"):
    """body_mode: 'full' (the real kernel), 'noden' (denominator path
    removed) or 'dmaonly' (input DMAs + output copies only) — the reduced
    bodies are timing-ablation probes, not functional kernels."""
    import concourse.tile as tile
    from concourse import bacc, mybir

    F32 = mybir.dt.float32
    BF16 = mybir.dt.bfloat16
    Tmax = max(extents)
    Wmax = _blob_width(Tmax)

    nc = bacc.Bacc("TRN2")

    ins = {}
    outs = {}
    for s, T in enumerate(extents):
        ins[f"blob{s}"] = nc.dram_tensor(
            f"blob{s}", [128, _blob_width(T)], BF16, kind="ExternalInput"
        )
        outs[f"num{s}"] = nc.dram_tensor(f"num{s}", [128, QCHUNK], F32, kind="ExternalOutput")
        if body_mode != "full":
            outs[f"den{s}"] = nc.dram_tensor(f"den{s}", [1, QCHUNK], F32, kind="ExternalOutput")

    def emit_schedule(tc, pools):
        inp, epool, dpool, opool, dqool, ps_s, ps_o, ps_d = pools
        for s, T in [(s, T) for _ in range(repeat) for s, T in enumerate(extents)]:
            W = _blob_width(T)
            ng = (T + 1) // 2
            q_off, k_off, v_off, w_off, g_off = _blob_offsets(T)

            # One DMA per slot: dma_start issue costs ~1us on the SP queue
            # and HW ablation shows the DMA/issue skeleton dominates the
            # schedule, so fewer DMAs beats the earlier split-for-latency.
            blob = inp.tile([128, Wmax], BF16, tag="blob")
            nc.sync.dma_start(out=blob[:, :W], in_=ins[f"blob{s}"][:, :])
            qt = blob[:, q_off : q_off + QCHUNK]

            po = ps_o.tile([128, QCHUNK], F32, tag="po")
            pd = ps_d.tile([128, QCHUNK], F32, tag="pd")
            if body_mode == "dmaonly":
                osb = opool.tile([128, QCHUNK], F32, tag="osb")
                nc.vector.tensor_copy(osb, blob[:, 0:QCHUNK])
                nc.sync.dma_start(out=outs[f"num{s}"][:, :], in_=osb)
                dsb = dqool.tile([1, QCHUNK], F32, tag="dsb")
                nc.vector.tensor_copy(dsb, blob[0:1, 0:QCHUNK])
                nc.sync.dma_start(out=outs[f"den{s}"][:, :], in_=dsb)
                continue
            # Three-stage software pipeline over 2-tile groups:
            #   stage A (group g):   MM1 + exp
            #   stage B (group g-1): MM2 + DVE den pair-sum
            #   stage C (group g-2): den matmul (consumes the DVE result a
            #     full group later, so the in-order PE queue never stalls
            #     waiting for DVE)
            stage_b = None
            stage_c = None
            for g in range(ng + 2):
                if g < ng:
                    gtiles = list(range(2 * g, min(2 * g + 2, T)))
                    n = len(gtiles)
                    pss = ps_s.tile([128, 2 * QCHUNK], F32, tag="ps")
                    for j, t in enumerate(gtiles):
                        nc.tensor.matmul(
                            pss[:, j * QCHUNK : (j + 1) * QCHUNK],
                            blob[:, k_off + t * 128 : k_off + (t + 1) * 128],
                            qt,
                            start=True,
                            stop=True,
                        )
                    eg = epool.tile([128, 2 * QCHUNK], BF16, tag="eg")
                    nc.scalar.activation(
                        eg[:, : n * QCHUNK],
                        pss[:, : n * QCHUNK],
                        mybir.ActivationFunctionType.Exp,
                        scale=SCALE,
                    )
                    cur = (g, gtiles, eg)
                else:
                    cur = None
                if stage_b is not None:
                    pg, ptiles, peg = stage_b
                    # MM2 first: PE streams V-matmuls while DVE fuses the
                    # denominator pair-sum for the same group.
                    for j, t in enumerate(ptiles):
                        nc.tensor.matmul(
                            po,
                            blob[:, v_off + t * 128 : v_off + (t + 1) * 128],
                            peg[:, j * QCHUNK : (j + 1) * QCHUNK],
                            start=(t == 0),
                            stop=(t == T - 1),
                        )
                    if body_mode == "noden":
                        stage_c_next = None
                    elif len(ptiles) == 2:
                        dsum = dpool.tile([128, QCHUNK], BF16, tag="dsum")
                        nc.vector.scalar_tensor_tensor(
                            dsum,
                            peg[:, QCHUNK : 2 * QCHUNK],
                            blob[:, g_off + pg : g_off + pg + 1],
                            peg[:, 0:QCHUNK],
                            mybir.AluOpType.mult,
                            mybir.AluOpType.add,
                        )
                        mv = dsum[:, :]
                        stage_c_next = (pg, mv)
                    else:
                        mv = peg[:, 0:QCHUNK]
                        stage_c_next = (pg, mv)
                else:
                    stage_c_next = None
                if stage_c is not None:
                    pg, mv = stage_c
                    nc.tensor.matmul(
                        pd,
                        blob[:, w_off + pg * 128 : w_off + (pg + 1) * 128],
                        mv,
                        start=(pg == 0),
                        stop=(pg == ng - 1),
                    )
                stage_b = cur
                stage_c = stage_c_next
            # osb first: po completes with the last MM2, while the trailing
            # den matmul (stage C) still runs on PE; dsb follows.
            osb = opool.tile([128, QCHUNK], F32, tag="osb")
            if body_mode == "full":
                # Normalize on-device: pd holds den replicated on every
                # partition (broadcast-ones stationary), so po/pd is the
                # softmax-normalized output; one output DMA per slot
                # replaces copy+copy+2 DMAs. DVE can read only one PSUM
                # operand per instruction, so stage pd through SBUF.
                dsb = dqool.tile([128, QCHUNK], F32, tag="dsb")
                nc.vector.reciprocal(dsb, pd)
                nc.vector.tensor_tensor(
                    out=osb, in0=po, in1=dsb, op=mybir.AluOpType.mult
                )
            else:
                nc.vector.tensor_copy(osb, po)
            nc.sync.dma_start(out=outs[f"num{s}"][:, :], in_=osb)

    with tile.TileContext(nc) as tc:
        with (
            tc.tile_pool(name="inp", bufs=4) as inp,
            tc.tile_pool(name="epool", bufs=3) as epool,
            tc.tile_pool(name="dpool", bufs=3) as dpool,
            tc.tile_pool(name="opool", bufs=2) as opool,
            tc.tile_pool(name="dqool", bufs=2) as dqool,
            tc.tile_pool(name="ps_s", bufs=3, space="PSUM") as ps_s,
            tc.tile_pool(name="ps_o", bufs=1, space="PSUM") as ps_o,
            tc.tile_pool(name="ps_d", bufs=1, space="PSUM") as ps_d,
        ):
            pools = (inp, epool, dpool, opool, dqool, ps_s, ps_o, ps_d)
            if loop_n > 1:
                # The schedule body exceeds one 16KiB IRAM block on PE
                # (~267 instructions > 256), so without a branch hint the
                # For_i back-edge stalls ~3-4us on an IRAM fetch each
                # iteration; the hint arms the prefetcher for the target.
                with tc.For_i(
                    0,
                    loop_n,
                    1,
                    staggered_reset=staggered,
                    hint_engines=(mybir.EngineType.PE,),
                ):
                    emit_schedule(tc, pools)
            else:
                emit_schedule(tc, pools)

    nc.finalize()
    return nc


def _get_program(extents, repeat=1, loop_n=None, staggered=False, body_mode="full"):
    if loop_n is None:
        loop_n = _LOOP_N
    key = (tuple(extents), repeat, loop_n, staggered, body_mode)
    if key not in _programs:
        _programs[key] = _build_program(
            tuple(extents), repeat, loop_n, staggered, body_mode
        )
    return _programs[key]


def _shard_plan(vl):
    """64 (batch, q-chunk) shards sorted by key-tile count desc; slot s of
    core c runs shard rank s*8+c. Returns (shards, extents)."""
    tiles = [max(1, int(math.ceil(int(vl[b]) / 128.0))) for b in range(B)]
    shards = sorted(
        ((tiles[b], b, qc) for b in range(B) for qc in range(NQCHUNKS)),
        key=lambda x: (-x[0], x[1], x[2]),
    )
    extents = tuple(shards[s * NCORES][0] for s in range(NSLOTS))
    return shards, extents


def _make_in_maps(queries, keys, values, vl, shards, extents):
    import ml_dtypes

    bf16 = ml_dtypes.bfloat16

    # Per-batch cached layouts (bf16): K^T [128, 2048], V tile-major with
    # invalid rows zeroed [128, 2048], Q^T [128, 2048], and per-tile 0/1
    # validity columns z [128, 16].
    cache = {}

    def prep(b):
        if b not in cache:
            n = int(vl[b])
            kt = np.ascontiguousarray(keys[b].T).astype(bf16)
            vz = values[b].copy()
            vz[n:] = 0.0
            vt = np.ascontiguousarray(
                vz.reshape(L // 128, 128, D).transpose(1, 0, 2).reshape(128, L)
            ).astype(bf16)
            qt = np.ascontiguousarray(queries[b].T).astype(bf16)
            z = (np.arange(L, dtype=np.int64) < n).astype(np.float32)
            zc = np.ascontiguousarray(z.reshape(L // 128, 128).T).astype(bf16)
            cache[b] = (kt, vt, qt, zc)
        return cache[b]

    in_maps = [{} for _ in range(NCORES)]
    for s in range(NSLOTS):
        T = extents[s]
        ng = (T + 1) // 2
        W = _blob_width(T)
        q_off, k_off, v_off, w_off, g_off = _blob_offsets(T)
        for c in range(NCORES):
            _, b, qc = shards[s * NCORES + c]
            kt, vt, qt, zc = prep(b)
            blob = np.empty((128, W), dtype=bf16)
            blob[:, q_off : q_off + QCHUNK] = qt[:, qc * QCHUNK : (qc + 1) * QCHUNK]
            blob[:, k_off : k_off + 128 * T] = kt[:, : 128 * T]
            blob[:, v_off : v_off + 128 * T] = vt[:, : 128 * T]
            for g in range(ng):
                # den-matmul stationary: z of the even tile, broadcast to
                # the 128 stationary columns
                blob[:, w_off + g * 128 : w_off + (g + 1) * 128] = zc[:, 2 * g : 2 * g + 1]
                # gate column: z of the odd tile (0 when the pair has no
                # odd tile; the fused op is not emitted in that case)
                t1 = 2 * g + 1
                blob[:, g_off + g] = zc[:, t1] if t1 < T else 0
            in_maps[c][f"blob{s}"] = blob
    return in_maps


def kernel(queries, keys, values, valid_lens):
    from concourse.bass_utils import run_bass_kernel_spmd

    queries = np.ascontiguousarray(queries, dtype=np.float32)
    keys = np.ascontiguousarray(keys, dtype=np.float32)
    values = np.ascontiguousarray(values, dtype=np.float32)
    vl = np.asarray(valid_lens).astype(np.int64).clip(1, L)
    assert queries.shape == (B, L, D), queries.shape

    shards, extents = _shard_plan(vl)
    nc = _get_program(extents, _REPEAT)
    in_maps = _make_in_maps(queries, keys, values, vl, shards, extents)

    res = run_bass_kernel_spmd(nc, in_maps, core_ids=list(range(NCORES)), trace=_TRACE)
    globals()["_last_results"] = res

    out = np.empty((B, L, D), np.float32)
    for s in range(NSLOTS):
        for c in range(NCORES):
            _, b, qc = shards[s * NCORES + c]
            r = res.results[c]
            # num{s} is already softmax-normalized on device (po/pd)
            out[b, qc * QCHUNK : (qc + 1) * QCHUNK] = r[f"num{s}"].T
    return out


# revision 29
# speedup vs baseline: 1.5654x; 1.5654x over previous
"""Trainium2 Bass kernel for masked dot-product attention.

Problem: B=16, Lq=Lk=2048, d=128, fp32.
  scores = Q @ K^T / sqrt(d); mask key positions >= valid_len with -1e6;
  attn = softmax(scores, axis=-1); out = attn @ V.

Strategy
--------
Sharding (unchanged from the earlier revision): work is split over
(batch, query-quarter): 16 batches x 4 q-chunks of 512 = 64 shards, 8 per
core. A shard's device cost is proportional to ceil(valid_len/128) key
tiles, so shards are sorted by tile count and slot s of every core runs the
8 shards ranked [8s, 8s+8); the compiled program bakes per-slot key extents
E_s = max tile count in that rank band. Every core executes an identical
instruction stream (SPMD); all per-core variation lives in the data.

Device pipeline per slot (one 512-wide q-chunk, T = E_s key tiles), all
matmul operands bf16 (rel-err budget is 2e-2; measured ~4.5e-3):
  MM1:  S^T[k,q] = (K^T tile).T @ Q^T          (PE, k-tile stationary)
  exp:  E = exp(S^T / sqrt(d))                 (ACT, PSUM->SBUF bf16,
                                                2 k-tiles per instruction)
  den:  dsum_g = (E_odd * z_odd) + E_even      (DVE scalar_tensor_tensor,
                                                one op per 2-tile group)
        pd += (z_even x ones128).T @ dsum_g    (PE, one matmul per group)
  MM2:  po += V_tile.T-layout @ E              (PE accumulate over tiles)
MM1/exp run one group ahead of MM2/den (software pipeline, psum double
buffered) so PE, ACT and DVE overlap.

Masking is exact and purely data-driven (the instruction stream never
depends on a shard's actual valid_len, only on the baked extent):
  * V rows at k >= valid_len are zeroed on the host, so MM2 ignores
    invalid keys regardless of E's (garbage) values there.
  * The denominator gates E through per-tile 0/1 validity columns z_t:
    because validity is monotone (z_even >= z_odd pointwise),
    z_even*(E_even + z_odd*E_odd) == z_even*E_even + z_odd*E_odd exactly,
    so one fused DVE op + one matmul per pair computes the gated sum. The
    z columns ride in the packed input blob; fully-padded tiles get z=0.

All slot inputs (Q^T chunk | K^T | V tile-major | w-stationaries | gate
columns, all bf16) are packed host-side into ONE DRAM blob per slot,
transferred as two DMAs: the Q|K prefix first so MM1 starts before
V/w/gates arrive. Outputs (num [128,512] fp32, den [1,512] fp32) are
copied PSUM->SBUF on DVE and DMA'd out. Host computes out = (num^T /
den).T per shard. Softmax needs no max-subtraction: scores ~ N(0,1) so
exp() cannot overflow, and invalid columns are excluded exactly.
"""

import math

import numpy as np

B, L, D = 16, 2048, 128
NCORES = 8
QCHUNK = 512
NQCHUNKS = L // QCHUNK
NSLOTS = B * NQCHUNKS // NCORES  # 8
SCALE = 1.0 / math.sqrt(D)

_programs = {}

# Test hooks: _REPEAT>1 duplicates the whole slot schedule inside one NEFF
# (python-unrolled); _LOOP_N>1 wraps the schedule in a hardware For_i loop
# (small NEFF, device time scales ~linearly -> robust wall-clock deltas).
_TRACE = False
_REPEAT = 1
_LOOP_N = 1
_last_results = None


def _blob_width(T):
    ng = (T + 1) // 2
    return 256 * T + QCHUNK + 129 * ng


def _blob_offsets(T):
    """Blob column layout: Q^T chunk and K^T first (MM1 prerequisites, DMA'd
    as the first chunk so MM1 can start before V/w/gates arrive), then V
    tile-major, then den stationaries and gate columns."""
    ng = (T + 1) // 2
    q_off = 0
    k_off = QCHUNK
    v_off = QCHUNK + 128 * T
    w_off = v_off + 128 * T
    g_off = w_off + 128 * ng
    return q_off, k_off, v_off, w_off, g_off


def _build_program(extents, repeat=1, loop_n=1, staggered=False, body_mode="full"):
    """body_mode: 'full' (the real kernel), 'noden' (denominator path
    removed) or 'dmaonly' (input DMAs + output copies only) — the reduced
    bodies are timing-ablation probes, not functional kernels."""
    import concourse.tile as tile
    from concourse import bacc, mybir

    F32 = mybir.dt.float32
    BF16 = mybir.dt.bfloat16
    Tmax = max(extents)
    Wmax = _blob_width(Tmax)

    nc = bacc.Bacc("TRN2")

    ins = {}
    outs = {}
    for s, T in enumerate(extents):
        ins[f"blob{s}"] = nc.dram_tensor(
            f"blob{s}", [128, _blob_width(T)], BF16, kind="ExternalInput"
        )
        outs[f"num{s}"] = nc.dram_tensor(f"num{s}", [128, QCHUNK], F32, kind="ExternalOutput")
        outs[f"den{s}"] = nc.dram_tensor(f"den{s}", [1, QCHUNK], F32, kind="ExternalOutput")

    def emit_schedule(tc, pools):
        inp, epool, dpool, opool, dqool, ps_s, ps_o, ps_d = pools
        for s, T in [(s, T) for _ in range(repeat) for s, T in enumerate(extents)]:
            W = _blob_width(T)
            ng = (T + 1) // 2
            q_off, k_off, v_off, w_off, g_off = _blob_offsets(T)

            blob = inp.tile([128, Wmax], BF16, tag="blob")
            nc.sync.dma_start(out=blob[:, :v_off], in_=ins[f"blob{s}"][:, :v_off])
            nc.sync.dma_start(out=blob[:, v_off:W], in_=ins[f"blob{s}"][:, v_off:])
            qt = blob[:, q_off : q_off + QCHUNK]

            po = ps_o.tile([128, QCHUNK], F32, tag="po")
            pd = ps_d.tile([128, QCHUNK], F32, tag="pd")
            if body_mode == "dmaonly":
                osb = opool.tile([128, QCHUNK], F32, tag="osb")
                nc.vector.tensor_copy(osb, blob[:, 0:QCHUNK])
                nc.sync.dma_start(out=outs[f"num{s}"][:, :], in_=osb)
                dsb = dqool.tile([1, QCHUNK], F32, tag="dsb")
                nc.vector.tensor_copy(dsb, blob[0:1, 0:QCHUNK])
                nc.sync.dma_start(out=outs[f"den{s}"][:, :], in_=dsb)
                continue
            # Three-stage software pipeline over 2-tile groups:
            #   stage A (group g):   MM1 + exp
            #   stage B (group g-1): MM2 + DVE den pair-sum
            #   stage C (group g-2): den matmul (consumes the DVE result a
            #     full group later, so the in-order PE queue never stalls
            #     waiting for DVE)
            stage_b = None
            stage_c = None
            for g in range(ng + 2):
                if g < ng:
                    gtiles = list(range(2 * g, min(2 * g + 2, T)))
                    n = len(gtiles)
                    pss = ps_s.tile([128, 2 * QCHUNK], F32, tag="ps")
                    for j, t in enumerate(gtiles):
                        nc.tensor.matmul(
                            pss[:, j * QCHUNK : (j + 1) * QCHUNK],
                            blob[:, k_off + t * 128 : k_off + (t + 1) * 128],
                            qt,
                            start=True,
                            stop=True,
                        )
                    eg = epool.tile([128, 2 * QCHUNK], BF16, tag="eg")
                    nc.scalar.activation(
                        eg[:, : n * QCHUNK],
                        pss[:, : n * QCHUNK],
                        mybir.ActivationFunctionType.Exp,
                        scale=SCALE,
                    )
                    cur = (g, gtiles, eg)
                else:
                    cur = None
                if stage_b is not None:
                    pg, ptiles, peg = stage_b
                    # MM2 first: PE streams V-matmuls while DVE fuses the
                    # denominator pair-sum for the same group.
                    for j, t in enumerate(ptiles):
                        nc.tensor.matmul(
                            po,
                            blob[:, v_off + t * 128 : v_off + (t + 1) * 128],
                            peg[:, j * QCHUNK : (j + 1) * QCHUNK],
                            start=(t == 0),
                            stop=(t == T - 1),
                        )
                    if body_mode == "noden":
                        stage_c_next = None
                    elif len(ptiles) == 2:
                        dsum = dpool.tile([128, QCHUNK], BF16, tag="dsum")
                        nc.vector.scalar_tensor_tensor(
                            dsum,
                            peg[:, QCHUNK : 2 * QCHUNK],
                            blob[:, g_off + pg : g_off + pg + 1],
                            peg[:, 0:QCHUNK],
                            mybir.AluOpType.mult,
                            mybir.AluOpType.add,
                        )
                        mv = dsum[:, :]
                        stage_c_next = (pg, mv)
                    else:
                        mv = peg[:, 0:QCHUNK]
                        stage_c_next = (pg, mv)
                else:
                    stage_c_next = None
                if stage_c is not None:
                    pg, mv = stage_c
                    nc.tensor.matmul(
                        pd,
                        blob[:, w_off + pg * 128 : w_off + (pg + 1) * 128],
                        mv,
                        start=(pg == 0),
                        stop=(pg == ng - 1),
                    )
                stage_b = cur
                stage_c = stage_c_next
            # osb first: po completes with the last MM2, while the trailing
            # den matmul (stage C) still runs on PE; dsb follows.
            # osb first: po completes with the last MM2, while the trailing
            # den matmul (stage C) still runs on PE; dsb follows. (On-device
            # normalization via DVE reciprocal was tried and measured ~25us
            # SLOWER per schedule: the reciprocal is expensive on DVE and
            # sits on the serialized slot boundary. Host division wins.)
            osb = opool.tile([128, QCHUNK], F32, tag="osb")
            nc.vector.tensor_copy(osb, po)
            nc.sync.dma_start(out=outs[f"num{s}"][:, :], in_=osb)
            if body_mode == "full":
                dsb = dqool.tile([1, QCHUNK], F32, tag="dsb")
                nc.vector.tensor_copy(dsb, pd[0:1, :])
                nc.sync.dma_start(out=outs[f"den{s}"][:, :], in_=dsb)

    with tile.TileContext(nc) as tc:
        with (
            tc.tile_pool(name="inp", bufs=4) as inp,
            tc.tile_pool(name="epool", bufs=3) as epool,
            tc.tile_pool(name="dpool", bufs=3) as dpool,
            tc.tile_pool(name="opool", bufs=2) as opool,
            tc.tile_pool(name="dqool", bufs=2) as dqool,
            tc.tile_pool(name="ps_s", bufs=3, space="PSUM") as ps_s,
            tc.tile_pool(name="ps_o", bufs=1, space="PSUM") as ps_o,
            tc.tile_pool(name="ps_d", bufs=1, space="PSUM") as ps_d,
        ):
            pools = (inp, epool, dpool, opool, dqool, ps_s, ps_o, ps_d)
            if loop_n > 1:
                # The schedule body exceeds one 16KiB IRAM block on PE
                # (~267 instructions > 256), so without a branch hint the
                # For_i back-edge stalls ~3-4us on an IRAM fetch each
                # iteration; the hint arms the prefetcher for the target.
                with tc.For_i(
                    0,
                    loop_n,
                    1,
                    staggered_reset=staggered,
                    hint_engines=(mybir.EngineType.PE,),
                ):
                    emit_schedule(tc, pools)
            else:
                emit_schedule(tc, pools)

    nc.finalize()
    return nc


def _get_program(extents, repeat=1, loop_n=None, staggered=False, body_mode="full"):
    if loop_n is None:
        loop_n = _LOOP_N
    key = (tuple(extents), repeat, loop_n, staggered, body_mode)
    if key not in _programs:
        _programs[key] = _build_program(
            tuple(extents), repeat, loop_n, staggered, body_mode
        )
    return _programs[key]


def _shard_plan(vl):
    """64 (batch, q-chunk) shards sorted by key-tile count desc; slot s of
    core c runs shard rank s*8+c. Returns (shards, extents)."""
    tiles = [max(1, int(math.ceil(int(vl[b]) / 128.0))) for b in range(B)]
    shards = sorted(
        ((tiles[b], b, qc) for b in range(B) for qc in range(NQCHUNKS)),
        key=lambda x: (-x[0], x[1], x[2]),
    )
    extents = tuple(shards[s * NCORES][0] for s in range(NSLOTS))
    return shards, extents


def _make_in_maps(queries, keys, values, vl, shards, extents):
    import ml_dtypes

    bf16 = ml_dtypes.bfloat16

    # Per-batch cached layouts (bf16): K^T [128, 2048], V tile-major with
    # invalid rows zeroed [128, 2048], Q^T [128, 2048], and per-tile 0/1
    # validity columns z [128, 16].
    cache = {}

    def prep(b):
        if b not in cache:
            n = int(vl[b])
            kt = np.ascontiguousarray(keys[b].T).astype(bf16)
            vz = values[b].copy()
            vz[n:] = 0.0
            vt = np.ascontiguousarray(
                vz.reshape(L // 128, 128, D).transpose(1, 0, 2).reshape(128, L)
            ).astype(bf16)
            qt = np.ascontiguousarray(queries[b].T).astype(bf16)
            z = (np.arange(L, dtype=np.int64) < n).astype(np.float32)
            zc = np.ascontiguousarray(z.reshape(L // 128, 128).T).astype(bf16)
            cache[b] = (kt, vt, qt, zc)
        return cache[b]

    in_maps = [{} for _ in range(NCORES)]
    for s in range(NSLOTS):
        T = extents[s]
        ng = (T + 1) // 2
        W = _blob_width(T)
        q_off, k_off, v_off, w_off, g_off = _blob_offsets(T)
        for c in range(NCORES):
            _, b, qc = shards[s * NCORES + c]
            kt, vt, qt, zc = prep(b)
            blob = np.empty((128, W), dtype=bf16)
            blob[:, q_off : q_off + QCHUNK] = qt[:, qc * QCHUNK : (qc + 1) * QCHUNK]
            blob[:, k_off : k_off + 128 * T] = kt[:, : 128 * T]
            blob[:, v_off : v_off + 128 * T] = vt[:, : 128 * T]
            for g in range(ng):
                # den-matmul stationary: z of the even tile, broadcast to
                # the 128 stationary columns
                blob[:, w_off + g * 128 : w_off + (g + 1) * 128] = zc[:, 2 * g : 2 * g + 1]
                # gate column: z of the odd tile (0 when the pair has no
                # odd tile; the fused op is not emitted in that case)
                t1 = 2 * g + 1
                blob[:, g_off + g] = zc[:, t1] if t1 < T else 0
            in_maps[c][f"blob{s}"] = blob
    return in_maps


def kernel(queries, keys, values, valid_lens):
    from concourse.bass_utils import run_bass_kernel_spmd

    queries = np.ascontiguousarray(queries, dtype=np.float32)
    keys = np.ascontiguousarray(keys, dtype=np.float32)
    values = np.ascontiguousarray(values, dtype=np.float32)
    vl = np.asarray(valid_lens).astype(np.int64).clip(1, L)
    assert queries.shape == (B, L, D), queries.shape

    shards, extents = _shard_plan(vl)
    nc = _get_program(extents, _REPEAT)
    in_maps = _make_in_maps(queries, keys, values, vl, shards, extents)

    res = run_bass_kernel_spmd(nc, in_maps, core_ids=list(range(NCORES)), trace=_TRACE)
    globals()["_last_results"] = res

    out = np.empty((B, L, D), np.float32)
    for s in range(NSLOTS):
        for c in range(NCORES):
            _, b, qc = shards[s * NCORES + c]
            r = res.results[c]
            num = r[f"num{s}"]  # [128, QCHUNK]
            den = r[f"den{s}"]  # [1, QCHUNK]
            out[b, qc * QCHUNK : (qc + 1) * QCHUNK] = (num / den).T
    return out
